# revision 34
# baseline (speedup 1.0000x reference)
"""W8A8 quantized Llama MLP, tensor-parallel across 8 Trainium2 NeuronCores.

Sharding (per the TP hint): column-shard gate_up (each core owns 1376 gate +
1376 up channels, kept paired), row-shard down_proj (each core owns the
matching 1376 contraction rows), AllGather + local max for the per-token
dynamic requant scale (AllReduce-max is broken in this runtime), chunked
ReduceScatter(add) for the output partials.

Wire format is int8 for x and both weight matrices (cast to bf16 on-device
via SWDGE DMA-cast); x ships token-sharded and is AllGathered on-device;
the output returns as bf16 and is upcast on the host. This cuts host->device
bytes ~12x vs replicating bf16 weights on every core.

Per core: bf16 matmuls (int8 exact in bf16, fp32 PSUM), fp32 dequant +
fused-SiLU epilogue, per-token requant via magic-number round-to-nearest-even,
DMA-transpose of y_q for the down-projection contraction.
"""

import numpy as np
import ml_dtypes

T, H, I = 4096, 4096, 11008
N_CORES = 8
IC = I // N_CORES            # 1376 intermediate channels per core
ICP = 1408                   # padded to 11 * 128
KI = ICP // 128              # 11 contraction tiles for MM2
K1 = H // 128                # 32 contraction tiles for MM1
NB = 8                       # token blocks of 512
S = 4                        # 128-token tiles per block
WIN = [(0, 512), (512, 512), (1024, 352)]   # channel windows within IC
# packed gate+up weight layout: per partition, window-major [32 k][G cw | U cw]
PK_OFF = [0, 32 * 1024, 32 * 1024 + 32 * 1024]   # byte offsets of each window
WGU_ROW = 32 * 2752                               # per-partition row length
WD_ROW = 8 * KI * 512                             # packed w_down row: [hc][a][512]
HC = 8                       # output chunks of 512 over H
MAGIC = 12582912.0           # 1.5 * 2^23: fp32 RNE-to-integer magic constant
RG = [list(range(N_CORES))]
RS_CHUNKS = 4                # ReduceScatter chunks over the 4096 tokens

_bf16 = ml_dtypes.bfloat16

_prog_cache = {}


def _split_excess_waits(nc, mybir, bass_rust):
    """This walrus build allows only 1 sync-wait per instruction; hoist
    excess waits onto injected NOPs placed just before the instruction."""
    for f in nc.m.functions:
        for bb in f.blocks:
            insts = list(bb.instructions)
            out, changed = [], False
            for inst in insts:
                si = getattr(inst, "sync_info", None)
                if si is not None and si.on_wait is not None and len(si.on_wait) > 1:
                    waits = list(si.on_wait)
                    for w in waits[:-1]:
                        nop = bass_rust.InstNoOp(name=f"I-{nc.next_id()}", ins=[], outs=[])
                        nop.engine = inst.engine
                        nop.sync_info = mybir.SyncInfo(on_wait=[w], on_update=[])
                        out.append(nop)
                    inst.sync_info = mybir.SyncInfo(
                        on_wait=[waits[-1]], on_update=list(si.on_update or [])
                    )
                    changed = True
                out.append(inst)
            if changed:
                bb.instructions = out


def _build_program(reps=1, no_rs=False, no_xag=False, no_mag=False,
                   rs_chunks=RS_CHUNKS):
    import concourse.bass as bass
    import concourse.mybir as mybir
    import concourse.tile as tile
    import bass_rust
    from concourse.bass import ds, ts

    f32 = mybir.dt.float32
    bf = mybir.dt.bfloat16
    i8 = mybir.dt.int8
    AF = mybir.ActivationFunctionType
    ALU = mybir.AluOpType
    X = mybir.AxisListType.X

    nc = bass.Bass()
    xT_d = nc.dram_tensor("xTs", [128, K1, 512], i8, kind="ExternalInput")
    wgu_d = nc.dram_tensor("wguT", [128, WGU_ROW], i8, kind="ExternalInput")
    sgu_d = nc.dram_tensor("sgu", [2 * IC], f32, kind="ExternalInput")
    wd_d = nc.dram_tensor("wdT", [128, WD_ROW], i8, kind="ExternalInput")
    swd_d = nc.dram_tensor("swd", [H], f32, kind="ExternalInput")
    xs_d = nc.dram_tensor("xs", [128, 32], f32, kind="ExternalInput")
    out_d = nc.dram_tensor("out", [4, 128, H], bf, kind="ExternalOutput")
    ybuf_d = nc.dram_tensor("ybuf", [32, 128, IC], f32, kind="Internal")
    x_agT = nc.dram_tensor("x_agT", [N_CORES, 128, K1, 512], i8, kind="Internal",
                           addr_space="Shared")
    m_ag = nc.dram_tensor("m_ag", [N_CORES, 128, 32], f32, kind="Internal",
                          addr_space="Shared")

    with tile.TileContext(nc) as tc:
        with tc.tile_pool(name="consts", bufs=1) as consts, \
             tc.tile_pool(name="xpool", bufs=2) as xpool, \
             tc.tile_pool(name="wpool", bufs=2) as wpool, \
             tc.tile_pool(name="w2pool", bufs=3) as w2pool, \
             tc.tile_pool(name="gpool", bufs=3) as gpool, \
             tc.tile_pool(name="sigpool", bufs=4) as sigpool, \
             tc.tile_pool(name="upool", bufs=3) as upool, \
             tc.tile_pool(name="qpool", bufs=2) as qpool, \
             tc.tile_pool(name="ypool", bufs=2) as ypool, \
             tc.tile_pool(name="opool", bufs=3) as opool, \
             tc.tile_pool(name="ocpool", bufs=2) as ocpool, \
             tc.tile_pool(name="bpool", bufs=2) as bpool, \
             tc.tile_pool(name="psum", bufs=8, space="PSUM") as psum, \
             tc.tile_pool(name="dram", bufs=1, space="DRAM") as dram:

            # ------- resident constants -------
            xs_sb = consts.tile([128, 32], f32)
            nc.sync.dma_start(xs_sb[:], xs_d[:])
            sgB = consts.tile([128, 2 * IC], f32)
            nc.sync.dma_start(sgB[:], sgu_d[:][None, :].to_broadcast((128, 2 * IC)))
            m_all = consts.tile([128, 32, 3], f32)
            m_loc = consts.tile([128, 32], f32)
            m_sb = consts.tile([128, 32], f32)
            s2_sb = consts.tile([128, 32], f32)
            r_sb = consts.tile([128, 32], f32)

            # ------- DRAM bounce buffers -------
            xg_in = dram.tile([128, K1, 512], i8)
            m_in = dram.tile([128, 32], f32)
            part_d = dram.tile([T, H], bf)
            rs_d = dram.tile([512, H], bf)

            def emit_body():
              # ------- AllGather x (token-sharded, pre-transposed) -------
              if not no_xag:
                nc.sync.dma_start(xg_in[:], xT_d[:])
                nc.gpsimd.collective_compute(
                    "AllGather", ALU.bypass, replica_groups=RG,
                    ins=[xg_in.opt()], outs=[x_agT[:].opt()])

              # ------- Pass A: MM1 + dequant + SiLU*up + local absmax -------
              for b in range(NB):
                xT_sb = xpool.tile([128, K1, 512], bf, name="xT", tag="xT")
                nc.gpsimd.dma_start(xT_sb[:], x_agT[b])
                for wi, (c0, cw) in enumerate(WIN):
                    tcw = 2 * cw
                    psG = [psum.tile([128, 512], f32, name=f"psG{s}", tag="ps")
                           for s in range(S)]
                    psU = [psum.tile([128, 512], f32, name=f"psU{s}", tag="ps")
                           for s in range(S)]
                    for j in range(4):
                        wt = wpool.tile([128, 8 * 1024], bf, name="wg", tag="w")
                        nc.gpsimd.dma_start(
                            wt[:, ds(0, 8 * tcw)],
                            wgu_d[:, ds(PK_OFF[wi] + j * 8 * tcw, 8 * tcw)])
                        for kk in range(8):
                            k = j * 8 + kk
                            for s in range(S):
                                nc.tensor.matmul(
                                    psG[s][:, :cw],
                                    lhsT=xT_sb[:, k, ts(s, 128)],
                                    rhs=wt[:, ds(kk * tcw, cw)],
                                    start=(k == 0), stop=(k == K1 - 1))
                                nc.tensor.matmul(
                                    psU[s][:, :cw],
                                    lhsT=xT_sb[:, k, ts(s, 128)],
                                    rhs=wt[:, ds(kk * tcw + cw, cw)],
                                    start=(k == 0), stop=(k == K1 - 1))
                    for s in range(S):
                        t = b * 4 + s
                        xs_ap = xs_sb[:, t:t + 1]
                        g_t = gpool.tile([128, 512], f32, name="g_t", tag="g")
                        nc.vector.scalar_tensor_tensor(
                            g_t[:, :cw], psG[s][:, :cw], xs_ap, sgB[:, ds(c0, cw)],
                            ALU.mult, ALU.mult)
                        sg_t = sigpool.tile([128, 512], f32, name="sig", tag="sig")
                        nc.scalar.activation(sg_t[:, :cw], g_t[:, :cw], AF.Silu)
                        u_t = upool.tile([128, 512], f32, name="u_t", tag="u")
                        nc.vector.scalar_tensor_tensor(
                            u_t[:, :cw], psU[s][:, :cw], xs_ap,
                            sgB[:, ds(IC + c0, cw)], ALU.mult, ALU.mult)
                        nc.vector.tensor_tensor(
                            u_t[:, :cw], u_t[:, :cw], sg_t[:, :cw], ALU.mult)
                        nc.vector.tensor_reduce(
                            m_all[:, t, wi:wi + 1], u_t[:, :cw], axis=X,
                            op=ALU.max, apply_absolute_value=True)
                        nc.sync.dma_start(ybuf_d[t, :, ds(c0, cw)], u_t[:, :cw])

              # ------- global per-token scale: AllGather + local max -------
              # (AllReduce(max) is broken in this runtime — crashes the device)
              for t in range(32):
                nc.vector.tensor_reduce(
                    m_loc[:, t:t + 1], m_all[:, t, :], axis=X, op=ALU.max)
              if no_mag:
                nc.vector.tensor_copy(m_sb[:], m_loc[:])
              else:
                nc.sync.dma_start(m_in[:], m_loc[:])
                nc.gpsimd.collective_compute(
                    "AllGather", ALU.bypass, replica_groups=RG,
                    ins=[m_in.opt()], outs=[m_ag[:].opt()])
                for c in range(N_CORES):
                  mc = bpool.tile([128, 32], f32, name="mc", tag="mc")
                  nc.sync.dma_start(mc[:], m_ag[c])
                  if c == 0:
                    nc.vector.tensor_copy(m_sb[:], mc[:])
                  else:
                    nc.vector.tensor_tensor(m_sb[:], m_sb[:], mc[:], ALU.max)
              nc.vector.tensor_scalar(
                  s2_sb[:], m_sb[:], 1e-8, 1.0 / 127.0, ALU.max, ALU.mult)
              nc.vector.reciprocal(r_sb[:], s2_sb[:])

              # ------- Pass B: requant, MM2, partials, ReduceScatter -------
              for b in range(NB):
                yqTb = [ypool.tile([128, KI, 128], bf, name=f"yqT{s}",
                                   tag=f"yqT{s}") for s in range(S)]
                for s in range(S):
                    t = b * 4 + s
                    for wi, (c0, cw) in enumerate(WIN):
                        ych = qpool.tile([128, 512], f32, name="ych", tag="ych")
                        nc.sync.dma_start(ych[:, :cw], ybuf_d[t, :, ds(c0, cw)])
                        t1 = qpool.tile([128, 512], f32, name="t1", tag="t1")
                        nc.scalar.activation(t1[:, :cw], ych[:, :cw], AF.Copy,
                                             bias=MAGIC, scale=r_sb[:, t:t + 1])
                        yq = qpool.tile([128, 512], bf, name="yq", tag="yq")
                        nc.vector.tensor_scalar(yq[:, :cw], t1[:, :cw], MAGIC,
                                                None, ALU.subtract)
                        nj = (cw + 127) // 128
                        if cw % 128:
                            nc.vector.memset(yq[:, cw:nj * 128], 0.0)
                        for j in range(nj):
                            nc.scalar.dma_start_transpose(
                                yqTb[s][:, wi * 4 + j, :], yq[:, ts(j, 128)])
                for hc in range(HC):
                    swdB = bpool.tile([128, 512], f32, name="swdB", tag="swdB")
                    nc.sync.dma_start(
                        swdB[:],
                        swd_d[ds(hc * 512, 512)][None, :].to_broadcast((128, 512)))
                    ps2 = [psum.tile([128, 512], f32, name=f"ps2_{s}", tag="ps")
                           for s in range(S)]
                    for jk in range(3):
                        kn = min(4, KI - jk * 4)
                        wt2 = w2pool.tile([128, 4 * 512], bf, name="wt2", tag="w2")
                        nc.gpsimd.dma_start(
                            wt2[:, ds(0, kn * 512)],
                            wd_d[:, ds(hc * (KI * 512) + jk * 4 * 512, kn * 512)])
                        for kk in range(kn):
                            ki = jk * 4 + kk
                            for s in range(S):
                                nc.tensor.matmul(
                                    ps2[s][:], lhsT=yqTb[s][:, ki, :],
                                    rhs=wt2[:, ds(kk * 512, 512)],
                                    start=(ki == 0), stop=(ki == KI - 1))
                    for s in range(S):
                        t = b * 4 + s
                        ot = opool.tile([128, 512], bf, name="ot", tag="ot")
                        nc.vector.scalar_tensor_tensor(
                            ot[:], ps2[s][:], s2_sb[:, t:t + 1], swdB[:],
                            ALU.mult, ALU.mult)
                        nc.sync.dma_start(
                            part_d[ds(t * 128, 128), ds(hc * 512, 512)], ot[:])
                bpc = NB // rs_chunks          # token blocks per RS chunk
                if (b + 1) % bpc == 0 and not no_rs:
                    g = b // bpc
                    rows = bpc * 512
                    nc.gpsimd.collective_compute(
                        "ReduceScatter", ALU.add, replica_groups=RG,
                        ins=[part_d[ds(g * rows, rows), :].opt()],
                        outs=[rs_d[ds(g * (rows // 8), rows // 8), :].opt()])
                    for q in range(rows // 1024):
                        gg = g * (rows // 1024) + q
                        nc.sync.dma_start(out_d[gg], rs_d[ds(gg * 128, 128), :])

            for _ in range(reps):
                emit_body()

    import concourse.mybir as mybir2
    _split_excess_waits(nc, mybir2, bass_rust)
    return nc


def _get_nc():
    if "nc" not in _prog_cache:
        _prog_cache["nc"] = _build_program()
    return _prog_cache["nc"]


def _t8(a):
    """Fast int8 2D transpose via int64 two-pass: [R, C] -> [C, R], C % 8 == 0."""
    r, c = a.shape
    a64 = np.ascontiguousarray(a).view(np.int64)          # [R, C//8]
    b = np.ascontiguousarray(a64.T)                       # [C//8, R] (8B moves)
    return np.ascontiguousarray(
        b.view(np.int8).reshape(c // 8, r, 8).transpose(0, 2, 1)).reshape(c, r)


def make_in_maps(inputs):
    """Shard + pack the full inputs into per-core input maps (int8 wire)."""
    x_q = np.asarray(inputs["x_q"]).astype(np.int8)
    xsf = np.asarray(inputs["x_scale"], dtype=np.float32)
    xs_host = np.ascontiguousarray(xsf.reshape(32, 128).T)
    wgu = np.asarray(inputs["w_gate_up"]).astype(np.int8)
    sgu = np.asarray(inputs["s_w_gate_up"], dtype=np.float32)
    wd = np.asarray(inputs["w_down"]).astype(np.int8)
    swd = np.ascontiguousarray(np.asarray(inputs["s_w_down"], dtype=np.float32))
    def pk(rows):
        # [n, H] channel rows -> partition-major [128, 32, n] (contiguous)
        return np.ascontiguousarray(
            _t8(np.ascontiguousarray(rows)).reshape(K1, 128, -1)
            .transpose(1, 0, 2))

    maps = []
    for c in range(N_CORES):
        g0 = c * IC
        # x: [512 tok, H] -> [H, 512] -> partition-major [128, 32, 512]
        xTs = np.ascontiguousarray(
            _t8(x_q[c * 512:(c + 1) * 512]).reshape(K1, 128, 512)
            .transpose(1, 0, 2))
        # gate_up: per window, interleave [G cw | U cw] per (partition, k-tile)
        wins = []
        for (c0, cw) in WIN:
            gp = pk(wgu[g0 + c0:g0 + c0 + cw])          # [128, 32, cw]
            up = pk(wgu[I + g0 + c0:I + g0 + c0 + cw])  # [128, 32, cw]
            wins.append(np.concatenate([gp, up], axis=2).reshape(128, -1))
        wguT_c = np.ascontiguousarray(np.concatenate(wins, axis=1))
        sgu_c = np.ascontiguousarray(
            np.concatenate([sgu[g0:g0 + IC], sgu[I + g0:I + g0 + IC]]))
        # w_down: [ICP, H] -> [128, hc, a, 512] packed row per partition
        wdT_c = np.zeros((ICP, H), np.int8)
        wdT_c[:IC] = _t8(np.ascontiguousarray(wd[:, g0:g0 + IC]))
        wd_pk = np.ascontiguousarray(
            wdT_c.reshape(KI, 128, HC, 512).transpose(1, 2, 0, 3)
            .reshape(128, WD_ROW))
        maps.append({"xTs": xTs, "wguT": wguT_c, "sgu": sgu_c,
                     "wdT": wd_pk, "swd": swd, "xs": xs_host})
    return maps


def assemble_out(per_core_outs, rs_chunks=RS_CHUNKS):
    """per_core_outs[c] is the [4, 128, H] bf16 'out' tensor of core c."""
    rows = T // rs_chunks        # tokens per RS chunk
    r8 = rows // N_CORES         # rows owned per core per chunk
    out = np.empty((T, H), np.float32)
    for c in range(N_CORES):
        r = np.asarray(per_core_outs[c]).astype(np.float32).reshape(512, H)
        for g in range(rs_chunks):
            out[g * rows + c * r8:g * rows + (c + 1) * r8] = \
                r[g * r8:(g + 1) * r8]
    return out


def kernel(x_q, x_scale, w_gate_up, s_w_gate_up, w_down, s_w_down):
    from concourse.bass_utils import run_bass_kernel_spmd

    nc = _get_nc()
    in_maps = make_in_maps(dict(
        x_q=x_q, x_scale=x_scale, w_gate_up=w_gate_up,
        s_w_gate_up=s_w_gate_up, w_down=w_down, s_w_down=s_w_down))
    res = run_bass_kernel_spmd(nc, in_maps, core_ids=list(range(N_CORES)),
                               trace=False)
    return assemble_out([r["out"] for r in res.results])


# revision 35
# speedup vs baseline: 1.1495x; 1.1495x over previous
"""W8A8 quantized Llama MLP, tensor-parallel across 8 Trainium2 NeuronCores.

Sharding (per the TP hint): column-shard gate_up (each core owns 1376 gate +
1376 up channels, kept paired), row-shard down_proj (each core owns the
matching 1376 contraction rows), AllGather + local max for the per-token
dynamic requant scale (AllReduce-max is broken in this runtime), chunked
ReduceScatter(add) for the output partials.

Wire format is int8 for x and both weight matrices (cast to bf16 on-device
via SWDGE DMA-cast); x ships token-sharded and is AllGathered on-device;
the output returns as bf16 and is upcast on the host. This cuts host->device
bytes ~12x vs replicating bf16 weights on every core.

Per core: bf16 matmuls (int8 exact in bf16, fp32 PSUM), fp32 dequant +
fused-SiLU epilogue, per-token requant via magic-number round-to-nearest-even,
DMA-transpose of y_q for the down-projection contraction.
"""

import numpy as np
import ml_dtypes

T, H, I = 4096, 4096, 11008
N_CORES = 8
IC = I // N_CORES            # 1376 intermediate channels per core
ICP = 1408                   # padded to 11 * 128
KI = ICP // 128              # 11 contraction tiles for MM2
K1 = H // 128                # 32 contraction tiles for MM1
NB = 8                       # token blocks of 512
S = 4                        # 128-token tiles per block
WIN = [(0, 512), (512, 512), (1024, 352)]   # channel windows within IC
HC = 8                       # output chunks of 512 over H
MAGIC = 12582912.0           # 1.5 * 2^23: fp32 RNE-to-integer magic constant
RG = [list(range(N_CORES))]
RS_CHUNKS = 4                # ReduceScatter chunks over the 4096 tokens

_bf16 = ml_dtypes.bfloat16

_prog_cache = {}


def _split_excess_waits(nc, mybir, bass_rust):
    """This walrus build allows only 1 sync-wait per instruction; hoist
    excess waits onto injected NOPs placed just before the instruction."""
    for f in nc.m.functions:
        for bb in f.blocks:
            insts = list(bb.instructions)
            out, changed = [], False
            for inst in insts:
                si = getattr(inst, "sync_info", None)
                if si is not None and si.on_wait is not None and len(si.on_wait) > 1:
                    waits = list(si.on_wait)
                    for w in waits[:-1]:
                        nop = bass_rust.InstNoOp(name=f"I-{nc.next_id()}", ins=[], outs=[])
                        nop.engine = inst.engine
                        nop.sync_info = mybir.SyncInfo(on_wait=[w], on_update=[])
                        out.append(nop)
                    inst.sync_info = mybir.SyncInfo(
                        on_wait=[waits[-1]], on_update=list(si.on_update or [])
                    )
                    changed = True
                out.append(inst)
            if changed:
                bb.instructions = out


def _build_program(reps=1, no_rs=False, no_xag=False, no_mag=False,
                   rs_chunks=RS_CHUNKS):
    import concourse.bass as bass
    import concourse.mybir as mybir
    import concourse.tile as tile
    import bass_rust
    from concourse.bass import ds, ts

    f32 = mybir.dt.float32
    bf = mybir.dt.bfloat16
    i8 = mybir.dt.int8
    AF = mybir.ActivationFunctionType
    ALU = mybir.AluOpType
    X = mybir.AxisListType.X

    nc = bass.Bass()
    xT_d = nc.dram_tensor("xTs", [H, 512], i8, kind="ExternalInput")
    wgu_d = nc.dram_tensor("wguT", [H, 2 * IC], i8, kind="ExternalInput")
    sgu_d = nc.dram_tensor("sgu", [2 * IC], f32, kind="ExternalInput")
    wd_d = nc.dram_tensor("wdT", [ICP, H], i8, kind="ExternalInput")
    swd_d = nc.dram_tensor("swd", [H], f32, kind="ExternalInput")
    xs_d = nc.dram_tensor("xs", [128, 32], f32, kind="ExternalInput")
    out_d = nc.dram_tensor("out", [4, 128, H], bf, kind="ExternalOutput")
    ybuf_d = nc.dram_tensor("ybuf", [32, 128, IC], f32, kind="Internal")
    x_agT = nc.dram_tensor("x_agT", [N_CORES, H, 512], i8, kind="Internal",
                           addr_space="Shared")
    m_ag = nc.dram_tensor("m_ag", [N_CORES, 128, 32], f32, kind="Internal",
                          addr_space="Shared")

    with tile.TileContext(nc) as tc:
        with tc.tile_pool(name="consts", bufs=1) as consts, \
             tc.tile_pool(name="xpool", bufs=2) as xpool, \
             tc.tile_pool(name="wpool", bufs=3) as wpool, \
             tc.tile_pool(name="gpool", bufs=3) as gpool, \
             tc.tile_pool(name="sigpool", bufs=6) as sigpool, \
             tc.tile_pool(name="upool", bufs=3) as upool, \
             tc.tile_pool(name="qpool", bufs=2) as qpool, \
             tc.tile_pool(name="ypool", bufs=2) as ypool, \
             tc.tile_pool(name="opool", bufs=3) as opool, \
             tc.tile_pool(name="ocpool", bufs=2) as ocpool, \
             tc.tile_pool(name="bpool", bufs=2) as bpool, \
             tc.tile_pool(name="psum", bufs=8, space="PSUM") as psum, \
             tc.tile_pool(name="dram", bufs=1, space="DRAM") as dram:

            # ------- resident constants -------
            xs_sb = consts.tile([128, 32], f32)
            nc.sync.dma_start(xs_sb[:], xs_d[:])
            sgB = consts.tile([128, 2 * IC], f32)
            nc.sync.dma_start(sgB[:], sgu_d[:][None, :].to_broadcast((128, 2 * IC)))
            m_all = consts.tile([128, 32, 3], f32)
            m_loc = consts.tile([128, 32], f32)
            m_sb = consts.tile([128, 32], f32)
            s2_sb = consts.tile([128, 32], f32)
            r_sb = consts.tile([128, 32], f32)

            # ------- DRAM bounce buffers -------
            xg_in = dram.tile([H, 512], i8)
            m_in = dram.tile([128, 32], f32)
            part_d = dram.tile([T, H], bf)
            rs_d = dram.tile([512, H], bf)

            def emit_body():
              # ------- AllGather x (token-sharded, pre-transposed) -------
              if not no_xag:
                nc.sync.dma_start(xg_in[:], xT_d[:])
                nc.gpsimd.collective_compute(
                    "AllGather", ALU.bypass, replica_groups=RG,
                    ins=[xg_in.opt()], outs=[x_agT[:].opt()])

              # ------- Pass A: MM1 + dequant + SiLU*up + local absmax -------
              for b in range(NB):
                xT_sb = xpool.tile([128, K1, 512], bf, name="xT", tag="xT")
                nc.gpsimd.dma_start(
                    xT_sb[:], x_agT[b].rearrange("(a p) t -> p a t", p=128))
                for wi, (c0, cw) in enumerate(WIN):
                    psG = [psum.tile([128, 512], f32, name=f"psG{s}", tag="ps")
                           for s in range(S)]
                    for j in range(4):
                        wg = wpool.tile([128, 8, 512], bf, name="wg", tag="w")
                        nc.gpsimd.dma_start(
                            wg[:, :, :cw],
                            wgu_d[ds(j * 1024, 1024), ds(c0, cw)]
                            .rearrange("(a p) n -> p a n", p=128))
                        for kk in range(8):
                            k = j * 8 + kk
                            for s in range(S):
                                nc.tensor.matmul(
                                    psG[s][:, :cw],
                                    lhsT=xT_sb[:, k, ts(s, 128)],
                                    rhs=wg[:, kk, :cw],
                                    start=(k == 0), stop=(k == K1 - 1))
                    sigs = []
                    for s in range(S):
                        xs_ap = xs_sb[:, b * 4 + s:b * 4 + s + 1]
                        g_t = gpool.tile([128, 512], f32, name="g_t", tag="g")
                        nc.vector.scalar_tensor_tensor(
                            g_t[:, :cw], psG[s][:, :cw], xs_ap, sgB[:, ds(c0, cw)],
                            ALU.mult, ALU.mult)
                        sg_t = sigpool.tile([128, 512], f32, name="sig", tag="sig")
                        nc.scalar.activation(sg_t[:, :cw], g_t[:, :cw], AF.Silu)
                        sigs.append(sg_t)
                    psU = [psum.tile([128, 512], f32, name=f"psU{s}", tag="ps")
                           for s in range(S)]
                    for j in range(4):
                        wu = wpool.tile([128, 8, 512], bf, name="wu", tag="w")
                        nc.gpsimd.dma_start(
                            wu[:, :, :cw],
                            wgu_d[ds(j * 1024, 1024), ds(IC + c0, cw)]
                            .rearrange("(a p) n -> p a n", p=128))
                        for kk in range(8):
                            k = j * 8 + kk
                            for s in range(S):
                                nc.tensor.matmul(
                                    psU[s][:, :cw],
                                    lhsT=xT_sb[:, k, ts(s, 128)],
                                    rhs=wu[:, kk, :cw],
                                    start=(k == 0), stop=(k == K1 - 1))
                    for s in range(S):
                        t = b * 4 + s
                        xs_ap = xs_sb[:, t:t + 1]
                        u_t = upool.tile([128, 512], f32, name="u_t", tag="u")
                        nc.vector.scalar_tensor_tensor(
                            u_t[:, :cw], psU[s][:, :cw], xs_ap,
                            sgB[:, ds(IC + c0, cw)], ALU.mult, ALU.mult)
                        nc.vector.tensor_tensor(
                            u_t[:, :cw], u_t[:, :cw], sigs[s][:, :cw], ALU.mult)
                        nc.vector.tensor_reduce(
                            m_all[:, t, wi:wi + 1], u_t[:, :cw], axis=X,
                            op=ALU.max, apply_absolute_value=True)
                        nc.sync.dma_start(ybuf_d[t, :, ds(c0, cw)], u_t[:, :cw])

              # ------- global per-token scale: AllGather + local max -------
              # (AllReduce(max) is broken in this runtime — crashes the device)
              for t in range(32):
                nc.vector.tensor_reduce(
                    m_loc[:, t:t + 1], m_all[:, t, :], axis=X, op=ALU.max)
              if no_mag:
                nc.vector.tensor_copy(m_sb[:], m_loc[:])
              else:
                nc.sync.dma_start(m_in[:], m_loc[:])
                nc.gpsimd.collective_compute(
                    "AllGather", ALU.bypass, replica_groups=RG,
                    ins=[m_in.opt()], outs=[m_ag[:].opt()])
                for c in range(N_CORES):
                  mc = bpool.tile([128, 32], f32, name="mc", tag="mc")
                  nc.sync.dma_start(mc[:], m_ag[c])
                  if c == 0:
                    nc.vector.tensor_copy(m_sb[:], mc[:])
                  else:
                    nc.vector.tensor_tensor(m_sb[:], m_sb[:], mc[:], ALU.max)
              nc.vector.tensor_scalar(
                  s2_sb[:], m_sb[:], 1e-8, 1.0 / 127.0, ALU.max, ALU.mult)
              nc.vector.reciprocal(r_sb[:], s2_sb[:])

              # ------- Pass B: requant, MM2, partials, ReduceScatter -------
              for b in range(NB):
                yqTb = [ypool.tile([128, KI, 128], bf, name=f"yqT{s}",
                                   tag=f"yqT{s}") for s in range(S)]
                for s in range(S):
                    t = b * 4 + s
                    for wi, (c0, cw) in enumerate(WIN):
                        ych = qpool.tile([128, 512], f32, name="ych", tag="ych")
                        nc.sync.dma_start(ych[:, :cw], ybuf_d[t, :, ds(c0, cw)])
                        t1 = qpool.tile([128, 512], f32, name="t1", tag="t1")
                        nc.scalar.activation(t1[:, :cw], ych[:, :cw], AF.Copy,
                                             bias=MAGIC, scale=r_sb[:, t:t + 1])
                        yq = qpool.tile([128, 512], bf, name="yq", tag="yq")
                        nc.vector.tensor_scalar(yq[:, :cw], t1[:, :cw], MAGIC,
                                                None, ALU.subtract)
                        nj = (cw + 127) // 128
                        if cw % 128:
                            nc.vector.memset(yq[:, cw:nj * 128], 0.0)
                        for j in range(nj):
                            nc.scalar.dma_start_transpose(
                                yqTb[s][:, wi * 4 + j, :], yq[:, ts(j, 128)])
                for hc in range(HC):
                    swdB = bpool.tile([128, 512], f32, name="swdB", tag="swdB")
                    nc.sync.dma_start(
                        swdB[:],
                        swd_d[ds(hc * 512, 512)][None, :].to_broadcast((128, 512)))
                    ps2 = [psum.tile([128, 512], f32, name=f"ps2_{s}", tag="ps")
                           for s in range(S)]
                    for jk in range(3):
                        kn = min(4, KI - jk * 4)
                        wt2 = wpool.tile([128, 4, 512], bf, name="wt2", tag="w")
                        nc.gpsimd.dma_start(
                            wt2[:, :kn, :],
                            wd_d[ds(jk * 512, kn * 128), ds(hc * 512, 512)]
                            .rearrange("(a p) n -> p a n", p=128))
                        for kk in range(kn):
                            ki = jk * 4 + kk
                            for s in range(S):
                                nc.tensor.matmul(
                                    ps2[s][:], lhsT=yqTb[s][:, ki, :],
                                    rhs=wt2[:, kk, :],
                                    start=(ki == 0), stop=(ki == KI - 1))
                    for s in range(S):
                        t = b * 4 + s
                        ot = opool.tile([128, 512], bf, name="ot", tag="ot")
                        nc.vector.scalar_tensor_tensor(
                            ot[:], ps2[s][:], s2_sb[:, t:t + 1], swdB[:],
                            ALU.mult, ALU.mult)
                        nc.sync.dma_start(
                            part_d[ds(t * 128, 128), ds(hc * 512, 512)], ot[:])
                bpc = NB // rs_chunks          # token blocks per RS chunk
                if (b + 1) % bpc == 0 and not no_rs:
                    g = b // bpc
                    rows = bpc * 512
                    nc.gpsimd.collective_compute(
                        "ReduceScatter", ALU.add, replica_groups=RG,
                        ins=[part_d[ds(g * rows, rows), :].opt()],
                        outs=[rs_d[ds(g * (rows // 8), rows // 8), :].opt()])
                    for q in range(rows // 1024):
                        gg = g * (rows // 1024) + q
                        nc.sync.dma_start(out_d[gg], rs_d[ds(gg * 128, 128), :])

            for _ in range(reps):
                emit_body()

    import concourse.mybir as mybir2
    _split_excess_waits(nc, mybir2, bass_rust)
    return nc


def _get_nc():
    if "nc" not in _prog_cache:
        _prog_cache["nc"] = _build_program()
    return _prog_cache["nc"]


def _t8(a):
    """Fast int8 2D transpose via int64 two-pass: [R, C] -> [C, R], C % 8 == 0."""
    r, c = a.shape
    a64 = np.ascontiguousarray(a).view(np.int64)          # [R, C//8]
    b = np.ascontiguousarray(a64.T)                       # [C//8, R] (8B moves)
    return np.ascontiguousarray(
        b.view(np.int8).reshape(c // 8, r, 8).transpose(0, 2, 1)).reshape(c, r)


def make_in_maps(inputs):
    """Shard + pack the full inputs into per-core input maps (int8 wire)."""
    x_q = np.asarray(inputs["x_q"]).astype(np.int8)
    xsf = np.asarray(inputs["x_scale"], dtype=np.float32)
    xs_host = np.ascontiguousarray(xsf.reshape(32, 128).T)
    wgu = np.asarray(inputs["w_gate_up"]).astype(np.int8)
    sgu = np.asarray(inputs["s_w_gate_up"], dtype=np.float32)
    wd = np.asarray(inputs["w_down"]).astype(np.int8)
    swd = np.ascontiguousarray(np.asarray(inputs["s_w_down"], dtype=np.float32))
    maps = []
    for c in range(N_CORES):
        g0 = c * IC
        xTs = _t8(x_q[c * 512:(c + 1) * 512])
        wgu_c = np.concatenate([wgu[g0:g0 + IC], wgu[I + g0:I + g0 + IC]], axis=0)
        wguT_c = _t8(wgu_c)
        sgu_c = np.ascontiguousarray(
            np.concatenate([sgu[g0:g0 + IC], sgu[I + g0:I + g0 + IC]]))
        wdT_c = np.zeros((ICP, H), np.int8)
        wdT_c[:IC] = _t8(np.ascontiguousarray(wd[:, g0:g0 + IC]))
        maps.append({"xTs": xTs, "wguT": wguT_c, "sgu": sgu_c,
                     "wdT": wdT_c, "swd": swd, "xs": xs_host})
    return maps


def assemble_out(per_core_outs, rs_chunks=RS_CHUNKS):
    """per_core_outs[c] is the [4, 128, H] bf16 'out' tensor of core c."""
    rows = T // rs_chunks        # tokens per RS chunk
    r8 = rows // N_CORES         # rows owned per core per chunk
    out = np.empty((T, H), np.float32)
    for c in range(N_CORES):
        r = np.asarray(per_core_outs[c]).astype(np.float32).reshape(512, H)
        for g in range(rs_chunks):
            out[g * rows + c * r8:g * rows + (c + 1) * r8] = \
                r[g * r8:(g + 1) * r8]
    return out


def kernel(x_q, x_scale, w_gate_up, s_w_gate_up, w_down, s_w_down):
    from concourse.bass_utils import run_bass_kernel_spmd

    nc = _get_nc()
    in_maps = make_in_maps(dict(
        x_q=x_q, x_scale=x_scale, w_gate_up=w_gate_up,
        s_w_gate_up=s_w_gate_up, w_down=w_down, s_w_down=s_w_down))
    res = run_bass_kernel_spmd(nc, in_maps, core_ids=list(range(N_CORES)),
                               trace=False)
    return assemble_out([r["out"] for r in res.results])
